# revision 91
# baseline (speedup 1.0000x reference)
"""Trainium2 Bass kernel for nn_Decoder (LSTM decoder, B=131072, H=64, 12 steps).

Data-parallel across 8 NeuronCores (batch sharded, weights replicated).

Math folding (host side, exact algebra):
  x_t = rel_{t-1} @ W_emb.T + b_emb enters gates only through W_ih @ x_t, so
    W_eff = W_hh + (W_ih @ W_emb) @ W_pos
    b_eff = b_ih + b_hh + W_ih @ b_emb + (W_ih @ W_emb) @ b_pos
  and the hot recurrence is gates_t = W_eff @ h_{t-1} + b_eff (t >= 1), with
  step 0 using W_hh on h_init plus (W_ih @ W_emb) @ obs_rel and a bias without
  the b_pos term.

Device layout: hidden-major, two batch strips packed in the 128 partitions
(rows 0:64 = strip A hidden, 64:128 = strip B hidden) so every engine op runs
full 128 lanes; gate matmuls use block-diagonal weights (K=128).

Engine assignment (cost-model driven): the Activation engine is the hard
bottleneck (5 table ops/step, dtype-independent cost), so all elementwise
tensor ops run on the Pool engine and activations are 2048 wide to halve ACT
per-instruction overhead. h-state, activations and gate weights are bf16
(fits SBUF, 2x smaller); the c accumulator stays fp32.

PSUM (8 banks) is managed manually as one persistent [128, 4096] tile with a
ring allocator over sub-regions; subtile dependency tracking provides the
hazards. Gates normally double-buffer 2048-wide; while a pos phase is
accumulating in banks 6/7, gates run 1024-wide triple-buffered on banks 0-5
so pos and gates never block each other.

Positions for all 12 steps are produced per batch-group by 13 accumulating
matmuls per 512-column chunk into a one-bank psum region (t in the partition
dim), cycling banks so chunk k+1 overlaps chunk k's copy-out:
  rel_t  = W_pos @ h_t                      (+ b_pos via copy bias)
  curr_t = obs + W_pos @ sum_{tau<=t} h_tau (+ (t+1) b_pos via copy bias)
Each group's pos phase is deferred into the next pair's early steps (or the
final step's shadow) so its matmuls hide under gate activations. The packed
[96, COLS] result is DMA'd out per chunk and unpacked on host.
"""

import numpy as np

PRED = 12
H = 64
B = 131072
NCORES = 8
BC = B // NCORES          # 16384 batch per core
COLS = BC // 2            # 8192 columns (2 strips per column)
GC = 2048                 # columns per group
NG = COLS // GC           # 4 groups
NCH = GC // 512           # 512-wide matmul chunks per group
NT = 32                   # packing tiles per core (512 batch each)
FT = COLS // NT           # 256 cols per packing tile

F32 = np.float32

_CACHE = {}


def _build_program():
    import concourse.mybir as mybir
    from concourse import bacc
    from concourse.tile import TileContext
    from contextlib import ExitStack

    f32 = mybir.dt.float32
    bf16 = mybir.dt.bfloat16
    AF = mybir.ActivationFunctionType

    nc = bacc.Bacc()

    h0p = nc.dram_tensor("h0p", [128, COLS], bf16, kind="ExternalInput")
    c0p = nc.dram_tensor("c0p", [128, COLS], f32, kind="ExternalInput")
    obsrel = nc.dram_tensor("obsrel", [4, COLS], bf16, kind="ExternalInput")
    obsp = nc.dram_tensor("obsp", [4, COLS], bf16, kind="ExternalInput")
    wg0 = nc.dram_tensor("wg0", [128, 512], bf16, kind="ExternalInput")
    wg = nc.dram_tensor("wg", [128, 512], bf16, kind="ExternalInput")
    wx = nc.dram_tensor("wx", [4, 512], bf16, kind="ExternalInput")
    b0 = nc.dram_tensor("b0", [128, 4], f32, kind="ExternalInput")
    bN = nc.dram_tensor("bN", [128, 4], f32, kind="ExternalInput")
    wpos = nc.dram_tensor("wpos", [128, PRED * 96], bf16, kind="ExternalInput")
    wposb = nc.dram_tensor("wposb", [4, 96], bf16, kind="ExternalInput")
    pbias = nc.dram_tensor("pbias", [96, 1], f32, kind="ExternalInput")
    posout = nc.dram_tensor("posout", [96, COLS], f32, kind="ExternalOutput")
    posout2 = nc.dram_tensor("posout2", [96, 2 * GC], f32, kind="ExternalOutput")

    with ExitStack() as ctx:
        tc = ctx.enter_context(TileContext(nc))
        const = ctx.enter_context(tc.tile_pool(name="const", bufs=1))
        hpool = ctx.enter_context(tc.tile_pool(name="hpool", bufs=28))
        cpool = ctx.enter_context(tc.tile_pool(name="cpool", bufs=2))
        stage = ctx.enter_context(tc.tile_pool(name="stage", bufs=2))
        obspool = ctx.enter_context(tc.tile_pool(name="obspool", bufs=2))
        ospool = ctx.enter_context(tc.tile_pool(name="ospool", bufs=4))
        gpsum = ctx.enter_context(tc.tile_pool(name="gpsum", bufs=1, space="PSUM"))

        # ---- resident weights ----
        # Critical-path loads (first gate matmul + activation) go first on the
        # SP queue; everything else is issued from the idle DVE queue so the
        # kernel's first ACT op isn't stuck behind serialized DMA issue.
        wg0_s = const.tile([128, 512], bf16)
        wg_s = const.tile([128, 512], bf16)
        wx_s = const.tile([4, 512], bf16)
        b0_s = const.tile([128, 4], f32)
        bN_s = const.tile([128, 4], f32)
        wpos_s = const.tile([128, PRED * 96], bf16)
        wposb_s = const.tile([4, 96], bf16)
        obs_s = const.tile([4, COLS], bf16)
        pbias_s = const.tile([96, 1], f32)
        nc.scalar.dma_start(wg0_s[:], wg0[:, :])
        nc.scalar.dma_start(b0_s[:], b0[:, :])
        nc.scalar.dma_start(wx_s[:], wx[:, :])
        nc.scalar.dma_start(wg_s[:], wg[:, :])
        nc.scalar.dma_start(bN_s[:], bN[:, :])

        # phase order i, g, f, o; gate indices in weight layout: i=0 f=1 g=2 o=3
        PHASES = ((0, AF.Sigmoid, "si"), (2, AF.Tanh, "gg"),
                  (1, AF.Sigmoid, "sf"), (3, AF.Sigmoid, "so"))

        def emit_group_loads(g):
            sl = slice(g * GC, (g + 1) * GC)
            hs0 = hpool.tile([128, GC], bf16, tag="hs", name=f"hs_g{g}_t0")
            ct = cpool.tile([128, GC], f32, tag="c", name=f"c_g{g}")
            orl = obspool.tile([4, GC], bf16, tag="orl", name=f"orl_g{g}")
            if g < 2:
                # first pair: chunked across two DMA queues so all four
                # chunks land with only two serialized issues per queue
                for ch in range(NCH):
                    s2 = slice(512 * ch, 512 * (ch + 1))
                    eng = nc.sync if ch % 2 == 0 else nc.gpsimd
                    eng.dma_start(hs0[:, s2], h0p[:, g * GC + 512 * ch:
                                                  g * GC + 512 * (ch + 1)])
            else:
                nc.sync.dma_start(hs0[:], h0p[:, sl])
            nc.sync.dma_start(orl[:], obsrel[:, sl])
            nc.sync.dma_start(ct[:], c0p[:, sl])
            return {"g": g, "hs": [hs0], "c": ct, "orl": orl}

        # ---- manually managed PSUM ----
        # One persistent 8-bank tile; sub-regions are handed out by a tiny
        # ring allocator. Subtile dependency tracking inserts the hazards.
        # Normally gates double-buffer 2048-wide ([0:4] / [4:8] banks). While
        # a pos phase accumulates in bank 7, gates run 1024-wide on banks 0-5
        # (triple buffered) so neither side ever blocks the other.
        PS = gpsum.tile([128, 4096], f32, name="PS")
        ring = {"mode": GC, "idx": 0}

        def set_mode(w):
            if ring["mode"] != w:
                ring["mode"] = w
                ring["idx"] = 0

        def psum_region():
            w = ring["mode"]
            runs = (0, 2048) if w == 2048 else (0, 1024, 2048)
            off = runs[ring["idx"] % len(runs)]
            ring["idx"] += 1
            return off, w

        def emit_step(st, t, split_h=False):
            g = st["g"]
            wsel = wg0_s if t == 0 else wg_s
            bsel = b0_s if t == 0 else bN_s
            acts = {}
            for gi, func, nm in PHASES:
                A = stage.tile([128, GC], bf16, tag=nm, name=f"{nm}_g{g}_t{t}")
                sub = 0
                while sub < GC:
                    off, w = psum_region()
                    P = PS[:, off:off + w]
                    for ch in range(w // 512):
                        s2 = slice(sub + 512 * ch, sub + 512 * (ch + 1))
                        p2 = slice(512 * ch, 512 * (ch + 1))
                        nc.tensor.matmul(
                            P[:, p2], lhsT=wsel[:, 128 * gi:128 * gi + 128],
                            rhs=st["hs"][t][:, s2], start=True, stop=(t != 0))
                        if t == 0:
                            nc.tensor.matmul(
                                P[:, p2], lhsT=wx_s[0:4, 128 * gi:128 * gi + 128],
                                rhs=st["orl"][0:4, s2], start=False, stop=True)
                    nc.scalar.activation(A[:, sub:sub + w], P[:], func,
                                         bias=bsel[:, gi:gi + 1])
                    sub += w
                acts[nm] = A
                if nm == "gg":
                    # t1 = sigmoid(i) * tanh(g), in place over si
                    nc.gpsimd.tensor_mul(acts["si"][:], acts["si"][:], A[:])
                elif nm == "sf":
                    c = st["c"]
                    nc.gpsimd.tensor_mul(c[:], A[:], c[:])
                    nc.gpsimd.tensor_add(c[:], c[:], acts["si"][:])
                    T = stage.tile([128, GC], bf16, tag="sf", name=f"tt_g{g}_t{t}")
                    nc.scalar.activation(T[:], c[:], AF.Tanh)
                    acts["tt"] = T
            hn = hpool.tile([128, GC], bf16, tag="hs", name=f"hs_g{g}_t{t + 1}")
            if split_h:
                # final h entirely on idle DVE: bypasses the Pool queue
                # (still draining the c-chain) so tail pos matmuls start
                # as soon as the activations finish
                nc.vector.tensor_mul(hn[:], acts["so"][:], acts["tt"][:])
            else:
                nc.gpsimd.tensor_mul(hn[:], acts["so"][:], acts["tt"][:])
            st["hs"].append(hn)

        def emit_pos(st, banks=(7, 6), t_lo=0, t_hi=PRED, with_obs=True):
            # emits pos passes for h_{t_lo+1}..h_{t_hi}; with t_lo == 0 the
            # obs pass and b_pos bias are included and the result goes to
            # posout; a t_lo > 0 remainder goes to posout2 (host adds them)
            g = st["g"]
            full = t_lo == 0
            for ch in range(NCH):
                S = ospool.tile([96, 512], f32, tag="os",
                                name=f"os_g{g}_{t_lo}_{ch}")
                # cycling banks lets chunk k+1's matmuls overlap chunk k's
                # copy-out instead of WAR-waiting on it
                bk = banks[ch % len(banks)]
                Pp = PS[:, 512 * bk:512 * bk + 512]
                s2 = slice(512 * ch, 512 * (ch + 1))
                for t in range(t_lo, t_hi):
                    nc.tensor.matmul(
                        Pp[0:96, :], lhsT=wpos_s[:, 96 * t:96 * t + 96],
                        rhs=st["hs"][t + 1][:, s2], start=(t == t_lo),
                        stop=(not with_obs and t == t_hi - 1))
                if with_obs:
                    nc.tensor.matmul(
                        Pp[0:96, :], lhsT=wposb_s[0:4, :],
                        rhs=obs_s[0:4, g * GC + 512 * ch:g * GC + 512 * (ch + 1)],
                        start=False, stop=True)
                if full:
                    # per-chunk copy + DMA pipeline with later chunks' matmuls;
                    # per-row b_pos terms ride along as a per-partition bias
                    nc.vector.tensor_scalar_add(S[:], Pp[0:96, :],
                                                pbias_s[:, 0:1])
                    nc.sync.dma_start(
                        posout[:, g * GC + 512 * ch:g * GC + 512 * (ch + 1)],
                        S[:])
                else:
                    nc.vector.tensor_copy(S[:], Pp[0:96, :])
                    base = (g - 2) * GC
                    nc.sync.dma_start(
                        posout2[:, base + 512 * ch:base + 512 * (ch + 1)],
                        S[:])

        states = [emit_group_loads(g) for g in range(NG)]
        # pos-phase constants: needed only from ~halfway in, load after the
        # state tiles so they never delay the recurrence
        nc.gpsimd.dma_start(wpos_s[:], wpos[:, :])
        nc.gpsimd.dma_start(wposb_s[:], wposb[:, :])
        nc.gpsimd.dma_start(obs_s[:], obsp[:, :])
        nc.gpsimd.dma_start(pbias_s[:], pbias[:, :])
        prev = None
        npairs = NG // 2
        for pair in range(npairs):
            stA, stB = states[2 * pair], states[2 * pair + 1]
            last = pair == npairs - 1
            for t in range(PRED):
                if last and t >= PRED - 2:
                    # final two steps run 1024-wide so banks 6/7 are free and
                    # the pos passes that don't need the last h can hide in
                    # their shadow
                    set_mode(1024)
                emit_step(stA, t, split_h=True)
                if last and t == PRED - 2:
                    emit_pos(stA, banks=(7,), t_hi=PRED - 1, with_obs=False)
                if last and t == PRED - 1:
                    emit_pos(stA, banks=(7,), t_lo=PRED - 1)
                emit_step(stB, t, split_h=True)
                if last and t == PRED - 2:
                    emit_pos(stB, banks=(6,), t_hi=PRED - 1, with_obs=False)
                # deferred pos of the previous pair runs in banks 6/7 while
                # this pair's early steps run 1024-wide on banks 0-5
                if prev is not None and t == 1:
                    set_mode(1024)
                    emit_pos(prev[0], banks=(7,))
                if prev is not None and t == 3:
                    emit_pos(prev[1], banks=(6,))
                if prev is not None and t == 4:
                    set_mode(2048)
                    prev = None
            prev = (stA, stB)
        emit_pos(prev[1], banks=(6, 7), t_lo=PRED - 1)

    nc.finalize()
    return nc


def _bf16(x):
    import concourse.mybir as mybir
    return np.asarray(x).astype(mybir.dt.np(mybir.dt.bfloat16))


def _prep_inputs(encoder_h, encoder_c, obs_final_pos, obs_final_pos_rel,
                 W_emb, b_emb, W_ih, W_hh, b_ih, b_hh, W_pos, b_pos):
    f64 = np.float64
    W_emb, b_emb = W_emb.astype(f64), b_emb.astype(f64)
    W_ih, W_hh = W_ih.astype(f64), W_hh.astype(f64)
    b_ih, b_hh = b_ih.astype(f64), b_hh.astype(f64)
    W_pos, b_pos = W_pos.astype(f64), b_pos.astype(f64)

    W_ihe = W_ih @ W_emb                     # [256, 2]
    W_eff = W_hh + W_ihe @ W_pos             # [256, 64]
    b_eff0 = b_ih + b_hh + W_ih @ b_emb      # [256]
    b_effN = b_eff0 + W_ihe @ b_pos          # [256]

    def blockdiag_gates(W):
        # -> [128, 4*128]: per gate gi, cols 128*gi:+128 = blockdiag(Wg.T, Wg.T)
        out = np.zeros((128, 512), f64)
        for gi in range(4):
            Wg = W[64 * gi:64 * gi + 64, :]  # [64(out), 64(in)]
            out[0:64, 128 * gi:128 * gi + 64] = Wg.T
            out[64:128, 128 * gi + 64:128 * gi + 128] = Wg.T
        return out

    wg0 = blockdiag_gates(W_hh)
    wg = blockdiag_gates(W_eff)

    wx = np.zeros((4, 512), f64)
    for gi in range(4):
        Wg = W_ihe[64 * gi:64 * gi + 64, :]  # [64, 2]
        wx[0:2, 128 * gi:128 * gi + 64] = Wg.T
        wx[2:4, 128 * gi + 64:128 * gi + 128] = Wg.T

    b0 = np.zeros((128, 4), f64)
    bN = np.zeros((128, 4), f64)
    for gi in range(4):
        b0[:, gi] = np.tile(b_eff0[64 * gi:64 * gi + 64], 2)
        bN[:, gi] = np.tile(b_effN[64 * gi:64 * gi + 64], 2)

    # pos weights: psum rows m = half*48 + t'*4 + s*2 + k
    wpos = np.zeros((128, PRED * 96), f64)
    for t in range(PRED):
        Wt = np.zeros((128, 96), f64)
        for s in range(2):
            for k in range(2):
                for tp in range(PRED):
                    if tp == t:
                        Wt[64 * s:64 * s + 64, 0 * 48 + tp * 4 + s * 2 + k] = W_pos[k, :]
                    if tp >= t:
                        Wt[64 * s:64 * s + 64, 1 * 48 + tp * 4 + s * 2 + k] = W_pos[k, :]
        wpos[:, 96 * t:96 * t + 96] = Wt

    # obs indicator rows (cum half only) + per-psum-row b_pos bias vector
    wposb = np.zeros((4, 96), f64)
    pbias = np.zeros((96, 1), f64)
    for tp in range(PRED):
        for s in range(2):
            for k in range(2):
                pbias[0 * 48 + tp * 4 + s * 2 + k, 0] = b_pos[k]
                pbias[1 * 48 + tp * 4 + s * 2 + k, 0] = (tp + 1) * b_pos[k]
                wposb[2 * s + k, 1 * 48 + tp * 4 + s * 2 + k] = 1.0

    h_all = np.asarray(encoder_h, F32)[0]     # [B, 64]
    c_all = np.asarray(encoder_c, F32)[0]
    obs = np.asarray(obs_final_pos, F32)      # [B, 2]
    obsr = np.asarray(obs_final_pos_rel, F32)

    def pack_state(X, rows):
        # per core: [BC, rows] -> [2*rows, COLS] with strip packing
        X = X.reshape(NCORES, NT, 2, FT, rows)
        return X.transpose(0, 2, 4, 1, 3).reshape(NCORES, 2 * rows, COLS)

    h0p = pack_state(h_all, H)
    c0p = pack_state(c_all, H)
    orl = pack_state(obsr, 2)
    obsp = pack_state(obs, 2)

    consts = dict(
        wg0=_bf16(wg0), wg=_bf16(wg), wx=_bf16(wx),
        b0=np.ascontiguousarray(b0, F32), bN=np.ascontiguousarray(bN, F32),
        wpos=_bf16(wpos), wposb=_bf16(wposb),
        pbias=np.ascontiguousarray(pbias, F32))

    in_maps = []
    for cid in range(NCORES):
        m = dict(consts)
        m["h0p"] = _bf16(h0p[cid])
        m["c0p"] = np.ascontiguousarray(c0p[cid])
        m["obsrel"] = _bf16(orl[cid])
        m["obsp"] = _bf16(obsp[cid])
        in_maps.append(m)
    return in_maps


def _unpack_outputs(results):
    rel_parts, cur_parts = [], []
    for cid in range(NCORES):
        po = results[cid]["posout"].copy()  # [96, COLS]
        # the last pair's h12-only pos remainder is a separate accumulator
        po[:, (NG - 2) * GC:] += results[cid]["posout2"]
        P = po.reshape(2, PRED, 2, 2, NT, FT)   # half, t, s, k, tile, j
        rel = P[0].transpose(0, 3, 1, 4, 2).reshape(PRED, BC, 2)
        cur = P[1].transpose(0, 3, 1, 4, 2).reshape(PRED, BC, 2)
        rel_parts.append(rel)
        cur_parts.append(cur)
    pred_rel = np.concatenate(rel_parts, axis=1)
    pred = np.concatenate(cur_parts, axis=1)
    return pred, pred_rel


def _run(in_maps, trace=False):
    from concourse import bass_utils
    if "nc" not in _CACHE:
        _CACHE["nc"] = _build_program()
    nc = _CACHE["nc"]
    res = bass_utils.run_bass_kernel_spmd(
        nc, in_maps, core_ids=list(range(NCORES)), trace=trace)
    return res


def kernel(**inputs):
    inputs = {k: np.asarray(v) for k, v in inputs.items()}
    in_maps = _prep_inputs(**inputs)
    res = _run(in_maps, trace=False)
    pred, pred_rel = _unpack_outputs(res.results)
    return pred.astype(F32), pred_rel.astype(F32)


# revision 94
# speedup vs baseline: 1.0024x; 1.0024x over previous
"""Trainium2 Bass kernel for nn_Decoder (LSTM decoder, B=131072, H=64, 12 steps).

Data-parallel across 8 NeuronCores (batch sharded, weights replicated).

Math folding (host side, exact algebra):
  x_t = rel_{t-1} @ W_emb.T + b_emb enters gates only through W_ih @ x_t, so
    W_eff = W_hh + (W_ih @ W_emb) @ W_pos
    b_eff = b_ih + b_hh + W_ih @ b_emb + (W_ih @ W_emb) @ b_pos
  and the hot recurrence is gates_t = W_eff @ h_{t-1} + b_eff (t >= 1), with
  step 0 using W_hh on h_init plus (W_ih @ W_emb) @ obs_rel and a bias without
  the b_pos term.

Device layout: hidden-major, two batch strips packed in the 128 partitions
(rows 0:64 = strip A hidden, 64:128 = strip B hidden) so every engine op runs
full 128 lanes; gate matmuls use block-diagonal weights (K=128).

Engine assignment (cost-model driven): the Activation engine is the hard
bottleneck (5 table ops/step, dtype-independent cost), so all elementwise
tensor ops run on the Pool engine and activations are 2048 wide to halve ACT
per-instruction overhead. h-state, activations and gate weights are bf16
(fits SBUF, 2x smaller); the c accumulator stays fp32.

PSUM (8 banks) is managed manually as one persistent [128, 4096] tile with a
ring allocator over sub-regions; subtile dependency tracking provides the
hazards. Gates normally double-buffer 2048-wide; while a pos phase is
accumulating in banks 6/7, gates run 1024-wide triple-buffered on banks 0-5
so pos and gates never block each other.

Positions for all 12 steps are produced per batch-group by 13 accumulating
matmuls per 512-column chunk into a one-bank psum region (t in the partition
dim), cycling banks so chunk k+1 overlaps chunk k's copy-out:
  rel_t  = W_pos @ h_t                      (+ b_pos via copy bias)
  curr_t = obs + W_pos @ sum_{tau<=t} h_tau (+ (t+1) b_pos via copy bias)
Each group's pos phase is deferred into the next pair's early steps (or the
final step's shadow) so its matmuls hide under gate activations. The packed
[96, COLS] result is DMA'd out per chunk and unpacked on host.
"""

import numpy as np

PRED = 12
H = 64
B = 131072
NCORES = 8
BC = B // NCORES          # 16384 batch per core
COLS = BC // 2            # 8192 columns (2 strips per column)
GC = 2048                 # columns per group
NG = COLS // GC           # 4 groups
NCH = GC // 512           # 512-wide matmul chunks per group
NT = 32                   # packing tiles per core (512 batch each)
FT = COLS // NT           # 256 cols per packing tile

F32 = np.float32

_CACHE = {}


def _build_program():
    import concourse.mybir as mybir
    from concourse import bacc
    from concourse.tile import TileContext
    from contextlib import ExitStack

    f32 = mybir.dt.float32
    bf16 = mybir.dt.bfloat16
    AF = mybir.ActivationFunctionType

    nc = bacc.Bacc()

    h0p = nc.dram_tensor("h0p", [128, COLS], bf16, kind="ExternalInput")
    c0p = nc.dram_tensor("c0p", [128, COLS], f32, kind="ExternalInput")
    obsrel = nc.dram_tensor("obsrel", [4, COLS], bf16, kind="ExternalInput")
    obsp = nc.dram_tensor("obsp", [4, COLS], bf16, kind="ExternalInput")
    wg0 = nc.dram_tensor("wg0", [128, 512], bf16, kind="ExternalInput")
    wg = nc.dram_tensor("wg", [128, 512], bf16, kind="ExternalInput")
    wx = nc.dram_tensor("wx", [4, 512], bf16, kind="ExternalInput")
    b0 = nc.dram_tensor("b0", [128, 4], f32, kind="ExternalInput")
    bN = nc.dram_tensor("bN", [128, 4], f32, kind="ExternalInput")
    wpos = nc.dram_tensor("wpos", [128, PRED * 96], bf16, kind="ExternalInput")
    wposb = nc.dram_tensor("wposb", [4, 96], bf16, kind="ExternalInput")
    pbias = nc.dram_tensor("pbias", [96, 1], f32, kind="ExternalInput")
    posout = nc.dram_tensor("posout", [96, COLS], f32, kind="ExternalOutput")
    posout2 = nc.dram_tensor("posout2", [96, 2 * GC], f32, kind="ExternalOutput")

    with ExitStack() as ctx:
        tc = ctx.enter_context(TileContext(nc))
        const = ctx.enter_context(tc.tile_pool(name="const", bufs=1))
        hpool = ctx.enter_context(tc.tile_pool(name="hpool", bufs=28))
        cpool = ctx.enter_context(tc.tile_pool(name="cpool", bufs=2))
        stage = ctx.enter_context(tc.tile_pool(name="stage", bufs=2))
        obspool = ctx.enter_context(tc.tile_pool(name="obspool", bufs=2))
        ospool = ctx.enter_context(tc.tile_pool(name="ospool", bufs=4))
        gpsum = ctx.enter_context(tc.tile_pool(name="gpsum", bufs=1, space="PSUM"))

        # ---- resident weights ----
        # Critical-path loads (first gate matmul + activation) go first on the
        # SP queue; everything else is issued from the idle DVE queue so the
        # kernel's first ACT op isn't stuck behind serialized DMA issue.
        wg0_s = const.tile([128, 512], bf16)
        wg_s = const.tile([128, 512], bf16)
        wx_s = const.tile([4, 512], bf16)
        b0_s = const.tile([128, 4], f32)
        bN_s = const.tile([128, 4], f32)
        wpos_s = const.tile([128, PRED * 96], bf16)
        wposb_s = const.tile([4, 96], bf16)
        obs_s = const.tile([4, COLS], bf16)
        pbias_s = const.tile([96, 1], f32)
        # only what the first activation needs issues from the ACT queue:
        # each DMA issue occupies the in-order ACT sequencer for ~667ns, so
        # anything else here delays the kernel's first activation
        nc.scalar.dma_start(wg0_s[:], wg0[:, :])
        nc.scalar.dma_start(b0_s[:], b0[:, :])
        nc.sync.dma_start(wx_s[:], wx[:, :])
        nc.gpsimd.dma_start(wg_s[:], wg[:, :])
        nc.gpsimd.dma_start(bN_s[:], bN[:, :])

        # phase order i, g, f, o; gate indices in weight layout: i=0 f=1 g=2 o=3
        PHASES = ((0, AF.Sigmoid, "si"), (2, AF.Tanh, "gg"),
                  (1, AF.Sigmoid, "sf"), (3, AF.Sigmoid, "so"))

        def emit_group_loads(g):
            sl = slice(g * GC, (g + 1) * GC)
            hs0 = hpool.tile([128, GC], bf16, tag="hs", name=f"hs_g{g}_t0")
            ct = cpool.tile([128, GC], f32, tag="c", name=f"c_g{g}")
            orl = obspool.tile([4, GC], bf16, tag="orl", name=f"orl_g{g}")
            if g < 2:
                # first pair: chunked across two DMA queues so all four
                # chunks land with only two serialized issues per queue
                for ch in range(NCH):
                    s2 = slice(512 * ch, 512 * (ch + 1))
                    eng = nc.sync if ch % 2 == 0 else nc.gpsimd
                    eng.dma_start(hs0[:, s2], h0p[:, g * GC + 512 * ch:
                                                  g * GC + 512 * (ch + 1)])
            else:
                nc.sync.dma_start(hs0[:], h0p[:, sl])
            nc.sync.dma_start(orl[:], obsrel[:, sl])
            nc.sync.dma_start(ct[:], c0p[:, sl])
            return {"g": g, "hs": [hs0], "c": ct, "orl": orl}

        # ---- manually managed PSUM ----
        # One persistent 8-bank tile; sub-regions are handed out by a tiny
        # ring allocator. Subtile dependency tracking inserts the hazards.
        # Normally gates double-buffer 2048-wide ([0:4] / [4:8] banks). While
        # a pos phase accumulates in bank 7, gates run 1024-wide on banks 0-5
        # (triple buffered) so neither side ever blocks the other.
        PS = gpsum.tile([128, 4096], f32, name="PS")

        # PE p-state warm-up: a garbage matmul at t~0 starts the ramp clock
        # so the first real gate matmuls run at full frequency. Reads a
        # never-written tile; writes the pos bank, which nothing reads.
        warm = const.tile([4, 512], bf16)
        nc.vector.memset(warm[:], 0)
        nc.tensor.matmul(PS[:, 3584:4096], lhsT=warm[0:4, 0:128],
                         rhs=warm[0:4, :], start=True, stop=True)

        ring = {"mode": GC, "idx": 0}

        def set_mode(w):
            if ring["mode"] != w:
                ring["mode"] = w
                ring["idx"] = 0

        def psum_region():
            w = ring["mode"]
            runs = (0, 2048) if w == 2048 else (0, 1024, 2048)
            off = runs[ring["idx"] % len(runs)]
            ring["idx"] += 1
            return off, w

        def emit_step(st, t, split_h=False):
            g = st["g"]
            wsel = wg0_s if t == 0 else wg_s
            bsel = b0_s if t == 0 else bN_s
            acts = {}
            for gi, func, nm in PHASES:
                A = stage.tile([128, GC], bf16, tag=nm, name=f"{nm}_g{g}_t{t}")
                sub = 0
                while sub < GC:
                    off, w = psum_region()
                    P = PS[:, off:off + w]
                    for ch in range(w // 512):
                        s2 = slice(sub + 512 * ch, sub + 512 * (ch + 1))
                        p2 = slice(512 * ch, 512 * (ch + 1))
                        nc.tensor.matmul(
                            P[:, p2], lhsT=wsel[:, 128 * gi:128 * gi + 128],
                            rhs=st["hs"][t][:, s2], start=True, stop=(t != 0))
                        if t == 0:
                            nc.tensor.matmul(
                                P[:, p2], lhsT=wx_s[0:4, 128 * gi:128 * gi + 128],
                                rhs=st["orl"][0:4, s2], start=False, stop=True)
                    nc.scalar.activation(A[:, sub:sub + w], P[:], func,
                                         bias=bsel[:, gi:gi + 1])
                    sub += w
                acts[nm] = A
                if nm == "gg":
                    # t1 = sigmoid(i) * tanh(g), in place over si
                    nc.gpsimd.tensor_mul(acts["si"][:], acts["si"][:], A[:])
                elif nm == "sf":
                    c = st["c"]
                    nc.gpsimd.tensor_mul(c[:], A[:], c[:])
                    nc.gpsimd.tensor_add(c[:], c[:], acts["si"][:])
                    T = stage.tile([128, GC], bf16, tag="sf", name=f"tt_g{g}_t{t}")
                    nc.scalar.activation(T[:], c[:], AF.Tanh)
                    acts["tt"] = T
            hn = hpool.tile([128, GC], bf16, tag="hs", name=f"hs_g{g}_t{t + 1}")
            if split_h:
                # final h entirely on idle DVE: bypasses the Pool queue
                # (still draining the c-chain) so tail pos matmuls start
                # as soon as the activations finish
                nc.vector.tensor_mul(hn[:], acts["so"][:], acts["tt"][:])
            else:
                nc.gpsimd.tensor_mul(hn[:], acts["so"][:], acts["tt"][:])
            st["hs"].append(hn)

        def emit_pos(st, banks=(7, 6), t_lo=0, t_hi=PRED, with_obs=True):
            # emits pos passes for h_{t_lo+1}..h_{t_hi}; with t_lo == 0 the
            # obs pass and b_pos bias are included and the result goes to
            # posout; a t_lo > 0 remainder goes to posout2 (host adds them)
            g = st["g"]
            full = t_lo == 0
            for ch in range(NCH):
                S = ospool.tile([96, 512], f32, tag="os",
                                name=f"os_g{g}_{t_lo}_{ch}")
                # cycling banks lets chunk k+1's matmuls overlap chunk k's
                # copy-out instead of WAR-waiting on it
                bk = banks[ch % len(banks)]
                Pp = PS[:, 512 * bk:512 * bk + 512]
                s2 = slice(512 * ch, 512 * (ch + 1))
                for t in range(t_lo, t_hi):
                    nc.tensor.matmul(
                        Pp[0:96, :], lhsT=wpos_s[:, 96 * t:96 * t + 96],
                        rhs=st["hs"][t + 1][:, s2], start=(t == t_lo),
                        stop=(not with_obs and t == t_hi - 1))
                if with_obs:
                    nc.tensor.matmul(
                        Pp[0:96, :], lhsT=wposb_s[0:4, :],
                        rhs=obs_s[0:4, g * GC + 512 * ch:g * GC + 512 * (ch + 1)],
                        start=False, stop=True)
                if full:
                    # per-chunk copy + DMA pipeline with later chunks' matmuls;
                    # per-row b_pos terms ride along as a per-partition bias
                    nc.vector.tensor_scalar_add(S[:], Pp[0:96, :],
                                                pbias_s[:, 0:1])
                    nc.sync.dma_start(
                        posout[:, g * GC + 512 * ch:g * GC + 512 * (ch + 1)],
                        S[:])
                else:
                    nc.vector.tensor_copy(S[:], Pp[0:96, :])
                    base = (g - 2) * GC
                    nc.sync.dma_start(
                        posout2[:, base + 512 * ch:base + 512 * (ch + 1)],
                        S[:])

        states = [emit_group_loads(g) for g in range(NG)]
        # pos-phase constants: needed only from ~halfway in, load after the
        # state tiles so they never delay the recurrence
        nc.gpsimd.dma_start(wpos_s[:], wpos[:, :])
        nc.gpsimd.dma_start(wposb_s[:], wposb[:, :])
        nc.gpsimd.dma_start(obs_s[:], obsp[:, :])
        nc.gpsimd.dma_start(pbias_s[:], pbias[:, :])
        prev = None
        npairs = NG // 2
        for pair in range(npairs):
            stA, stB = states[2 * pair], states[2 * pair + 1]
            last = pair == npairs - 1
            for t in range(PRED):
                if last and t >= PRED - 2:
                    # final two steps run 1024-wide so banks 6/7 are free and
                    # the pos passes that don't need the last h can hide in
                    # their shadow
                    set_mode(1024)
                emit_step(stA, t, split_h=True)
                if last and t == PRED - 2:
                    emit_pos(stA, banks=(7,), t_hi=PRED - 1, with_obs=False)
                if last and t == PRED - 1:
                    emit_pos(stA, banks=(7,), t_lo=PRED - 1)
                emit_step(stB, t, split_h=True)
                if last and t == PRED - 2:
                    emit_pos(stB, banks=(6,), t_hi=PRED - 1, with_obs=False)
                # deferred pos of the previous pair runs in banks 6/7 while
                # this pair's early steps run 1024-wide on banks 0-5
                if prev is not None and t == 1:
                    set_mode(1024)
                    emit_pos(prev[0], banks=(7,))
                if prev is not None and t == 3:
                    emit_pos(prev[1], banks=(6,))
                if prev is not None and t == 4:
                    set_mode(2048)
                    prev = None
            prev = (stA, stB)
        emit_pos(prev[1], banks=(6, 7), t_lo=PRED - 1)

    nc.finalize()
    return nc


def _bf16(x):
    import concourse.mybir as mybir
    return np.asarray(x).astype(mybir.dt.np(mybir.dt.bfloat16))


def _prep_inputs(encoder_h, encoder_c, obs_final_pos, obs_final_pos_rel,
                 W_emb, b_emb, W_ih, W_hh, b_ih, b_hh, W_pos, b_pos):
    f64 = np.float64
    W_emb, b_emb = W_emb.astype(f64), b_emb.astype(f64)
    W_ih, W_hh = W_ih.astype(f64), W_hh.astype(f64)
    b_ih, b_hh = b_ih.astype(f64), b_hh.astype(f64)
    W_pos, b_pos = W_pos.astype(f64), b_pos.astype(f64)

    W_ihe = W_ih @ W_emb                     # [256, 2]
    W_eff = W_hh + W_ihe @ W_pos             # [256, 64]
    b_eff0 = b_ih + b_hh + W_ih @ b_emb      # [256]
    b_effN = b_eff0 + W_ihe @ b_pos          # [256]

    def blockdiag_gates(W):
        # -> [128, 4*128]: per gate gi, cols 128*gi:+128 = blockdiag(Wg.T, Wg.T)
        out = np.zeros((128, 512), f64)
        for gi in range(4):
            Wg = W[64 * gi:64 * gi + 64, :]  # [64(out), 64(in)]
            out[0:64, 128 * gi:128 * gi + 64] = Wg.T
            out[64:128, 128 * gi + 64:128 * gi + 128] = Wg.T
        return out

    wg0 = blockdiag_gates(W_hh)
    wg = blockdiag_gates(W_eff)

    wx = np.zeros((4, 512), f64)
    for gi in range(4):
        Wg = W_ihe[64 * gi:64 * gi + 64, :]  # [64, 2]
        wx[0:2, 128 * gi:128 * gi + 64] = Wg.T
        wx[2:4, 128 * gi + 64:128 * gi + 128] = Wg.T

    b0 = np.zeros((128, 4), f64)
    bN = np.zeros((128, 4), f64)
    for gi in range(4):
        b0[:, gi] = np.tile(b_eff0[64 * gi:64 * gi + 64], 2)
        bN[:, gi] = np.tile(b_effN[64 * gi:64 * gi + 64], 2)

    # pos weights: psum rows m = half*48 + t'*4 + s*2 + k
    wpos = np.zeros((128, PRED * 96), f64)
    for t in range(PRED):
        Wt = np.zeros((128, 96), f64)
        for s in range(2):
            for k in range(2):
                for tp in range(PRED):
                    if tp == t:
                        Wt[64 * s:64 * s + 64, 0 * 48 + tp * 4 + s * 2 + k] = W_pos[k, :]
                    if tp >= t:
                        Wt[64 * s:64 * s + 64, 1 * 48 + tp * 4 + s * 2 + k] = W_pos[k, :]
        wpos[:, 96 * t:96 * t + 96] = Wt

    # obs indicator rows (cum half only) + per-psum-row b_pos bias vector
    wposb = np.zeros((4, 96), f64)
    pbias = np.zeros((96, 1), f64)
    for tp in range(PRED):
        for s in range(2):
            for k in range(2):
                pbias[0 * 48 + tp * 4 + s * 2 + k, 0] = b_pos[k]
                pbias[1 * 48 + tp * 4 + s * 2 + k, 0] = (tp + 1) * b_pos[k]
                wposb[2 * s + k, 1 * 48 + tp * 4 + s * 2 + k] = 1.0

    h_all = np.asarray(encoder_h, F32)[0]     # [B, 64]
    c_all = np.asarray(encoder_c, F32)[0]
    obs = np.asarray(obs_final_pos, F32)      # [B, 2]
    obsr = np.asarray(obs_final_pos_rel, F32)

    def pack_state(X, rows):
        # per core: [BC, rows] -> [2*rows, COLS] with strip packing
        X = X.reshape(NCORES, NT, 2, FT, rows)
        return X.transpose(0, 2, 4, 1, 3).reshape(NCORES, 2 * rows, COLS)

    h0p = pack_state(h_all, H)
    c0p = pack_state(c_all, H)
    orl = pack_state(obsr, 2)
    obsp = pack_state(obs, 2)

    consts = dict(
        wg0=_bf16(wg0), wg=_bf16(wg), wx=_bf16(wx),
        b0=np.ascontiguousarray(b0, F32), bN=np.ascontiguousarray(bN, F32),
        wpos=_bf16(wpos), wposb=_bf16(wposb),
        pbias=np.ascontiguousarray(pbias, F32))

    in_maps = []
    for cid in range(NCORES):
        m = dict(consts)
        m["h0p"] = _bf16(h0p[cid])
        m["c0p"] = np.ascontiguousarray(c0p[cid])
        m["obsrel"] = _bf16(orl[cid])
        m["obsp"] = _bf16(obsp[cid])
        in_maps.append(m)
    return in_maps


def _unpack_outputs(results):
    rel_parts, cur_parts = [], []
    for cid in range(NCORES):
        po = results[cid]["posout"].copy()  # [96, COLS]
        # the last pair's h12-only pos remainder is a separate accumulator
        po[:, (NG - 2) * GC:] += results[cid]["posout2"]
        P = po.reshape(2, PRED, 2, 2, NT, FT)   # half, t, s, k, tile, j
        rel = P[0].transpose(0, 3, 1, 4, 2).reshape(PRED, BC, 2)
        cur = P[1].transpose(0, 3, 1, 4, 2).reshape(PRED, BC, 2)
        rel_parts.append(rel)
        cur_parts.append(cur)
    pred_rel = np.concatenate(rel_parts, axis=1)
    pred = np.concatenate(cur_parts, axis=1)
    return pred, pred_rel


def _run(in_maps, trace=False):
    from concourse import bass_utils
    if "nc" not in _CACHE:
        _CACHE["nc"] = _build_program()
    nc = _CACHE["nc"]
    res = bass_utils.run_bass_kernel_spmd(
        nc, in_maps, core_ids=list(range(NCORES)), trace=trace)
    return res


def kernel(**inputs):
    inputs = {k: np.asarray(v) for k, v in inputs.items()}
    in_maps = _prep_inputs(**inputs)
    res = _run(in_maps, trace=False)
    pred, pred_rel = _unpack_outputs(res.results)
    return pred.astype(F32), pred_rel.astype(F32)


# revision 95
# speedup vs baseline: 1.0039x; 1.0015x over previous
"""Trainium2 Bass kernel for nn_Decoder (LSTM decoder, B=131072, H=64, 12 steps).

Data-parallel across 8 NeuronCores (batch sharded, weights replicated).

Math folding (host side, exact algebra):
  x_t = rel_{t-1} @ W_emb.T + b_emb enters gates only through W_ih @ x_t, so
    W_eff = W_hh + (W_ih @ W_emb) @ W_pos
    b_eff = b_ih + b_hh + W_ih @ b_emb + (W_ih @ W_emb) @ b_pos
  and the hot recurrence is gates_t = W_eff @ h_{t-1} + b_eff (t >= 1), with
  step 0 using W_hh on h_init plus (W_ih @ W_emb) @ obs_rel and a bias without
  the b_pos term.

Device layout: hidden-major, two batch strips packed in the 128 partitions
(rows 0:64 = strip A hidden, 64:128 = strip B hidden) so every engine op runs
full 128 lanes; gate matmuls use block-diagonal weights (K=128).

Engine assignment (cost-model driven): the Activation engine is the hard
bottleneck (5 table ops/step, dtype-independent cost), so all elementwise
tensor ops run on the Pool engine and activations are 2048 wide to halve ACT
per-instruction overhead. h-state, activations and gate weights are bf16
(fits SBUF, 2x smaller); the c accumulator stays fp32.

PSUM (8 banks) is managed manually as one persistent [128, 4096] tile with a
ring allocator over sub-regions; subtile dependency tracking provides the
hazards. Gates normally double-buffer 2048-wide; while a pos phase is
accumulating in banks 6/7, gates run 1024-wide triple-buffered on banks 0-5
so pos and gates never block each other.

Positions for all 12 steps are produced per batch-group by 13 accumulating
matmuls per 512-column chunk into a one-bank psum region (t in the partition
dim), cycling banks so chunk k+1 overlaps chunk k's copy-out:
  rel_t  = W_pos @ h_t                      (+ b_pos via copy bias)
  curr_t = obs + W_pos @ sum_{tau<=t} h_tau (+ (t+1) b_pos via copy bias)
Each group's pos phase is deferred into the next pair's early steps (or the
final step's shadow) so its matmuls hide under gate activations. The packed
[96, COLS] result is DMA'd out per chunk and unpacked on host.
"""

import numpy as np

PRED = 12
H = 64
B = 131072
NCORES = 8
BC = B // NCORES          # 16384 batch per core
COLS = BC // 2            # 8192 columns (2 strips per column)
GC = 2048                 # columns per group
NG = COLS // GC           # 4 groups
NCH = GC // 512           # 512-wide matmul chunks per group
NT = 32                   # packing tiles per core (512 batch each)
FT = COLS // NT           # 256 cols per packing tile

F32 = np.float32

_CACHE = {}


def _build_program():
    import concourse.mybir as mybir
    from concourse import bacc
    from concourse.tile import TileContext
    from contextlib import ExitStack

    f32 = mybir.dt.float32
    bf16 = mybir.dt.bfloat16
    AF = mybir.ActivationFunctionType

    nc = bacc.Bacc()

    h0p = nc.dram_tensor("h0p", [128, COLS], bf16, kind="ExternalInput")
    c0p = nc.dram_tensor("c0p", [128, COLS], f32, kind="ExternalInput")
    obsrel = nc.dram_tensor("obsrel", [4, COLS], bf16, kind="ExternalInput")
    obsp = nc.dram_tensor("obsp", [4, COLS], bf16, kind="ExternalInput")
    wg0 = nc.dram_tensor("wg0", [128, 512], bf16, kind="ExternalInput")
    wg = nc.dram_tensor("wg", [128, 512], bf16, kind="ExternalInput")
    wx = nc.dram_tensor("wx", [4, 512], bf16, kind="ExternalInput")
    b0 = nc.dram_tensor("b0", [128, 4], f32, kind="ExternalInput")
    bN = nc.dram_tensor("bN", [128, 4], f32, kind="ExternalInput")
    wpos = nc.dram_tensor("wpos", [128, PRED * 96], bf16, kind="ExternalInput")
    wposb = nc.dram_tensor("wposb", [4, 96], bf16, kind="ExternalInput")
    pbias = nc.dram_tensor("pbias", [96, 1], f32, kind="ExternalInput")
    posout = nc.dram_tensor("posout", [96, COLS], f32, kind="ExternalOutput")
    posout2 = nc.dram_tensor("posout2", [96, 2 * GC], f32, kind="ExternalOutput")

    with ExitStack() as ctx:
        tc = ctx.enter_context(TileContext(nc))
        const = ctx.enter_context(tc.tile_pool(name="const", bufs=1))
        hpool = ctx.enter_context(tc.tile_pool(name="hpool", bufs=28))
        cpool = ctx.enter_context(tc.tile_pool(name="cpool", bufs=2))
        stage = ctx.enter_context(tc.tile_pool(name="stage", bufs=2))
        obspool = ctx.enter_context(tc.tile_pool(name="obspool", bufs=2))
        ospool = ctx.enter_context(tc.tile_pool(name="ospool", bufs=4))
        gpsum = ctx.enter_context(tc.tile_pool(name="gpsum", bufs=1, space="PSUM"))

        # ---- resident weights ----
        # Critical-path loads (first gate matmul + activation) go first on the
        # SP queue; everything else is issued from the idle DVE queue so the
        # kernel's first ACT op isn't stuck behind serialized DMA issue.
        wg0_s = const.tile([128, 512], bf16)
        wg_s = const.tile([128, 512], bf16)
        wx_s = const.tile([4, 512], bf16)
        b0_s = const.tile([128, 4], f32)
        bN_s = const.tile([128, 4], f32)
        wpos_s = const.tile([128, PRED * 96], bf16)
        wposb_s = const.tile([4, 96], bf16)
        obs_s = const.tile([4, COLS], bf16)
        pbias_s = const.tile([96, 1], f32)
        # only what the first activation needs issues from the ACT queue:
        # each DMA issue occupies the in-order ACT sequencer for ~667ns, so
        # anything else here delays the kernel's first activation
        nc.scalar.dma_start(wg0_s[:], wg0[:, :])
        nc.scalar.dma_start(b0_s[:], b0[:, :])
        nc.sync.dma_start(wx_s[:], wx[:, :])
        nc.gpsimd.dma_start(wg_s[:], wg[:, :])
        nc.gpsimd.dma_start(bN_s[:], bN[:, :])

        # phase order i, g, f, o; gate indices in weight layout: i=0 f=1 g=2 o=3
        PHASES = ((0, AF.Sigmoid, "si"), (2, AF.Tanh, "gg"),
                  (1, AF.Sigmoid, "sf"), (3, AF.Sigmoid, "so"))

        def emit_group_loads(g):
            sl = slice(g * GC, (g + 1) * GC)
            hs0 = hpool.tile([128, GC], bf16, tag="hs", name=f"hs_g{g}_t0")
            ct = cpool.tile([128, GC], f32, tag="c", name=f"c_g{g}")
            orl = obspool.tile([4, GC], bf16, tag="orl", name=f"orl_g{g}")
            if g < 2:
                # first pair: chunked across two DMA queues so all four
                # chunks land with only two serialized issues per queue
                for ch in range(NCH):
                    s2 = slice(512 * ch, 512 * (ch + 1))
                    eng = nc.sync if ch % 2 == 0 else nc.gpsimd
                    eng.dma_start(hs0[:, s2], h0p[:, g * GC + 512 * ch:
                                                  g * GC + 512 * (ch + 1)])
            else:
                nc.sync.dma_start(hs0[:], h0p[:, sl])
            nc.sync.dma_start(orl[:], obsrel[:, sl])
            nc.sync.dma_start(ct[:], c0p[:, sl])
            return {"g": g, "hs": [hs0], "c": ct, "orl": orl}

        # ---- manually managed PSUM ----
        # One persistent 8-bank tile; sub-regions are handed out by a tiny
        # ring allocator. Subtile dependency tracking inserts the hazards.
        # Normally gates double-buffer 2048-wide ([0:4] / [4:8] banks). While
        # a pos phase accumulates in bank 7, gates run 1024-wide on banks 0-5
        # (triple buffered) so neither side ever blocks the other.
        PS = gpsum.tile([128, 4096], f32, name="PS")

        # PE p-state warm-up: a garbage matmul at t~0 starts the ramp clock
        # so the first real gate matmuls run at full frequency. Reads a
        # never-written tile; writes the pos bank, which nothing reads.
        warm = const.tile([4, 512], bf16)
        nc.vector.memset(warm[:], 0)
        nc.tensor.matmul(PS[:, 3584:4096], lhsT=warm[0:4, 0:128],
                         rhs=warm[0:4, :], start=True, stop=True)
        # ...and the ACT table load (~1.3us): a 1-element Sigmoid during the
        # input DMA latency loads the Sigmoid/Tanh table off the critical path
        nc.scalar.activation(warm[0:4, 0:1], warm[0:4, 0:1], AF.Sigmoid)

        ring = {"mode": GC, "idx": 0}

        def set_mode(w):
            if ring["mode"] != w:
                ring["mode"] = w
                ring["idx"] = 0

        def psum_region():
            w = ring["mode"]
            runs = (0, 2048) if w == 2048 else (0, 1024, 2048)
            off = runs[ring["idx"] % len(runs)]
            ring["idx"] += 1
            return off, w

        def emit_step(st, t, split_h=False):
            g = st["g"]
            wsel = wg0_s if t == 0 else wg_s
            bsel = b0_s if t == 0 else bN_s
            acts = {}
            for gi, func, nm in PHASES:
                A = stage.tile([128, GC], bf16, tag=nm, name=f"{nm}_g{g}_t{t}")
                sub = 0
                while sub < GC:
                    off, w = psum_region()
                    P = PS[:, off:off + w]
                    for ch in range(w // 512):
                        s2 = slice(sub + 512 * ch, sub + 512 * (ch + 1))
                        p2 = slice(512 * ch, 512 * (ch + 1))
                        nc.tensor.matmul(
                            P[:, p2], lhsT=wsel[:, 128 * gi:128 * gi + 128],
                            rhs=st["hs"][t][:, s2], start=True, stop=(t != 0))
                        if t == 0:
                            nc.tensor.matmul(
                                P[:, p2], lhsT=wx_s[0:4, 128 * gi:128 * gi + 128],
                                rhs=st["orl"][0:4, s2], start=False, stop=True)
                    nc.scalar.activation(A[:, sub:sub + w], P[:], func,
                                         bias=bsel[:, gi:gi + 1])
                    sub += w
                acts[nm] = A
                if nm == "gg":
                    # t1 = sigmoid(i) * tanh(g), in place over si
                    nc.gpsimd.tensor_mul(acts["si"][:], acts["si"][:], A[:])
                elif nm == "sf":
                    c = st["c"]
                    nc.gpsimd.tensor_mul(c[:], A[:], c[:])
                    nc.gpsimd.tensor_add(c[:], c[:], acts["si"][:])
                    T = stage.tile([128, GC], bf16, tag="sf", name=f"tt_g{g}_t{t}")
                    nc.scalar.activation(T[:], c[:], AF.Tanh)
                    acts["tt"] = T
            hn = hpool.tile([128, GC], bf16, tag="hs", name=f"hs_g{g}_t{t + 1}")
            if split_h:
                # final h entirely on idle DVE: bypasses the Pool queue
                # (still draining the c-chain) so tail pos matmuls start
                # as soon as the activations finish
                nc.vector.tensor_mul(hn[:], acts["so"][:], acts["tt"][:])
            else:
                nc.gpsimd.tensor_mul(hn[:], acts["so"][:], acts["tt"][:])
            st["hs"].append(hn)

        def emit_pos(st, banks=(7, 6), t_lo=0, t_hi=PRED, with_obs=True):
            # emits pos passes for h_{t_lo+1}..h_{t_hi}; with t_lo == 0 the
            # obs pass and b_pos bias are included and the result goes to
            # posout; a t_lo > 0 remainder goes to posout2 (host adds them)
            g = st["g"]
            full = t_lo == 0
            for ch in range(NCH):
                S = ospool.tile([96, 512], f32, tag="os",
                                name=f"os_g{g}_{t_lo}_{ch}")
                # cycling banks lets chunk k+1's matmuls overlap chunk k's
                # copy-out instead of WAR-waiting on it
                bk = banks[ch % len(banks)]
                Pp = PS[:, 512 * bk:512 * bk + 512]
                s2 = slice(512 * ch, 512 * (ch + 1))
                for t in range(t_lo, t_hi):
                    nc.tensor.matmul(
                        Pp[0:96, :], lhsT=wpos_s[:, 96 * t:96 * t + 96],
                        rhs=st["hs"][t + 1][:, s2], start=(t == t_lo),
                        stop=(not with_obs and t == t_hi - 1))
                if with_obs:
                    nc.tensor.matmul(
                        Pp[0:96, :], lhsT=wposb_s[0:4, :],
                        rhs=obs_s[0:4, g * GC + 512 * ch:g * GC + 512 * (ch + 1)],
                        start=False, stop=True)
                if full:
                    # per-chunk copy + DMA pipeline with later chunks' matmuls;
                    # per-row b_pos terms ride along as a per-partition bias
                    nc.vector.tensor_scalar_add(S[:], Pp[0:96, :],
                                                pbias_s[:, 0:1])
                    nc.sync.dma_start(
                        posout[:, g * GC + 512 * ch:g * GC + 512 * (ch + 1)],
                        S[:])
                else:
                    nc.vector.tensor_copy(S[:], Pp[0:96, :])
                    base = (g - 2) * GC
                    nc.sync.dma_start(
                        posout2[:, base + 512 * ch:base + 512 * (ch + 1)],
                        S[:])

        states = [emit_group_loads(g) for g in range(NG)]
        # pos-phase constants: needed only from ~halfway in, load after the
        # state tiles so they never delay the recurrence
        nc.gpsimd.dma_start(wpos_s[:], wpos[:, :])
        nc.gpsimd.dma_start(wposb_s[:], wposb[:, :])
        nc.gpsimd.dma_start(obs_s[:], obsp[:, :])
        nc.gpsimd.dma_start(pbias_s[:], pbias[:, :])
        prev = None
        npairs = NG // 2
        for pair in range(npairs):
            stA, stB = states[2 * pair], states[2 * pair + 1]
            last = pair == npairs - 1
            for t in range(PRED):
                if last and t >= PRED - 2:
                    # final two steps run 1024-wide so banks 6/7 are free and
                    # the pos passes that don't need the last h can hide in
                    # their shadow
                    set_mode(1024)
                emit_step(stA, t, split_h=True)
                if last and t == PRED - 2:
                    emit_pos(stA, banks=(7,), t_hi=PRED - 1, with_obs=False)
                if last and t == PRED - 1:
                    emit_pos(stA, banks=(7,), t_lo=PRED - 1)
                emit_step(stB, t, split_h=True)
                if last and t == PRED - 2:
                    emit_pos(stB, banks=(6,), t_hi=PRED - 1, with_obs=False)
                # deferred pos of the previous pair runs in banks 6/7 while
                # this pair's early steps run 1024-wide on banks 0-5
                if prev is not None and t == 1:
                    set_mode(1024)
                    emit_pos(prev[0], banks=(7,))
                if prev is not None and t == 3:
                    emit_pos(prev[1], banks=(6,))
                if prev is not None and t == 4:
                    set_mode(2048)
                    prev = None
            prev = (stA, stB)
        emit_pos(prev[1], banks=(6, 7), t_lo=PRED - 1)

    nc.finalize()
    return nc


def _bf16(x):
    import concourse.mybir as mybir
    return np.asarray(x).astype(mybir.dt.np(mybir.dt.bfloat16))


def _prep_inputs(encoder_h, encoder_c, obs_final_pos, obs_final_pos_rel,
                 W_emb, b_emb, W_ih, W_hh, b_ih, b_hh, W_pos, b_pos):
    f64 = np.float64
    W_emb, b_emb = W_emb.astype(f64), b_emb.astype(f64)
    W_ih, W_hh = W_ih.astype(f64), W_hh.astype(f64)
    b_ih, b_hh = b_ih.astype(f64), b_hh.astype(f64)
    W_pos, b_pos = W_pos.astype(f64), b_pos.astype(f64)

    W_ihe = W_ih @ W_emb                     # [256, 2]
    W_eff = W_hh + W_ihe @ W_pos             # [256, 64]
    b_eff0 = b_ih + b_hh + W_ih @ b_emb      # [256]
    b_effN = b_eff0 + W_ihe @ b_pos          # [256]

    def blockdiag_gates(W):
        # -> [128, 4*128]: per gate gi, cols 128*gi:+128 = blockdiag(Wg.T, Wg.T)
        out = np.zeros((128, 512), f64)
        for gi in range(4):
            Wg = W[64 * gi:64 * gi + 64, :]  # [64(out), 64(in)]
            out[0:64, 128 * gi:128 * gi + 64] = Wg.T
            out[64:128, 128 * gi + 64:128 * gi + 128] = Wg.T
        return out

    wg0 = blockdiag_gates(W_hh)
    wg = blockdiag_gates(W_eff)

    wx = np.zeros((4, 512), f64)
    for gi in range(4):
        Wg = W_ihe[64 * gi:64 * gi + 64, :]  # [64, 2]
        wx[0:2, 128 * gi:128 * gi + 64] = Wg.T
        wx[2:4, 128 * gi + 64:128 * gi + 128] = Wg.T

    b0 = np.zeros((128, 4), f64)
    bN = np.zeros((128, 4), f64)
    for gi in range(4):
        b0[:, gi] = np.tile(b_eff0[64 * gi:64 * gi + 64], 2)
        bN[:, gi] = np.tile(b_effN[64 * gi:64 * gi + 64], 2)

    # pos weights: psum rows m = half*48 + t'*4 + s*2 + k
    wpos = np.zeros((128, PRED * 96), f64)
    for t in range(PRED):
        Wt = np.zeros((128, 96), f64)
        for s in range(2):
            for k in range(2):
                for tp in range(PRED):
                    if tp == t:
                        Wt[64 * s:64 * s + 64, 0 * 48 + tp * 4 + s * 2 + k] = W_pos[k, :]
                    if tp >= t:
                        Wt[64 * s:64 * s + 64, 1 * 48 + tp * 4 + s * 2 + k] = W_pos[k, :]
        wpos[:, 96 * t:96 * t + 96] = Wt

    # obs indicator rows (cum half only) + per-psum-row b_pos bias vector
    wposb = np.zeros((4, 96), f64)
    pbias = np.zeros((96, 1), f64)
    for tp in range(PRED):
        for s in range(2):
            for k in range(2):
                pbias[0 * 48 + tp * 4 + s * 2 + k, 0] = b_pos[k]
                pbias[1 * 48 + tp * 4 + s * 2 + k, 0] = (tp + 1) * b_pos[k]
                wposb[2 * s + k, 1 * 48 + tp * 4 + s * 2 + k] = 1.0

    h_all = np.asarray(encoder_h, F32)[0]     # [B, 64]
    c_all = np.asarray(encoder_c, F32)[0]
    obs = np.asarray(obs_final_pos, F32)      # [B, 2]
    obsr = np.asarray(obs_final_pos_rel, F32)

    def pack_state(X, rows):
        # per core: [BC, rows] -> [2*rows, COLS] with strip packing
        X = X.reshape(NCORES, NT, 2, FT, rows)
        return X.transpose(0, 2, 4, 1, 3).reshape(NCORES, 2 * rows, COLS)

    h0p = pack_state(h_all, H)
    c0p = pack_state(c_all, H)
    orl = pack_state(obsr, 2)
    obsp = pack_state(obs, 2)

    consts = dict(
        wg0=_bf16(wg0), wg=_bf16(wg), wx=_bf16(wx),
        b0=np.ascontiguousarray(b0, F32), bN=np.ascontiguousarray(bN, F32),
        wpos=_bf16(wpos), wposb=_bf16(wposb),
        pbias=np.ascontiguousarray(pbias, F32))

    in_maps = []
    for cid in range(NCORES):
        m = dict(consts)
        m["h0p"] = _bf16(h0p[cid])
        m["c0p"] = np.ascontiguousarray(c0p[cid])
        m["obsrel"] = _bf16(orl[cid])
        m["obsp"] = _bf16(obsp[cid])
        in_maps.append(m)
    return in_maps


def _unpack_outputs(results):
    rel_parts, cur_parts = [], []
    for cid in range(NCORES):
        po = results[cid]["posout"].copy()  # [96, COLS]
        # the last pair's h12-only pos remainder is a separate accumulator
        po[:, (NG - 2) * GC:] += results[cid]["posout2"]
        P = po.reshape(2, PRED, 2, 2, NT, FT)   # half, t, s, k, tile, j
        rel = P[0].transpose(0, 3, 1, 4, 2).reshape(PRED, BC, 2)
        cur = P[1].transpose(0, 3, 1, 4, 2).reshape(PRED, BC, 2)
        rel_parts.append(rel)
        cur_parts.append(cur)
    pred_rel = np.concatenate(rel_parts, axis=1)
    pred = np.concatenate(cur_parts, axis=1)
    return pred, pred_rel


def _run(in_maps, trace=False):
    from concourse import bass_utils
    if "nc" not in _CACHE:
        _CACHE["nc"] = _build_program()
    nc = _CACHE["nc"]
    res = bass_utils.run_bass_kernel_spmd(
        nc, in_maps, core_ids=list(range(NCORES)), trace=trace)
    return res


def kernel(**inputs):
    inputs = {k: np.asarray(v) for k, v in inputs.items()}
    in_maps = _prep_inputs(**inputs)
    res = _run(in_maps, trace=False)
    pred, pred_rel = _unpack_outputs(res.results)
    return pred.astype(F32), pred_rel.astype(F32)
